# revision 20
# baseline (speedup 1.0000x reference)
"""Bass/Trainium2 kernel for the NaiveGNN message-passing problem.

Math (reference): h = emb @ W0 + b0 + sum_l (sum_j sigmoid(ee @ W1s[l])) @ W2s[l]
with ee[i,j] = [r_i - r_j, |r_i - r_j|^2].

Key identities:
  - The three layers share the ee tensor, so W1s concatenates to W1cat [4,96]
    and W2s to W2cat [96,64]: one fused pass with H=96 sigmoid features.
  - z[i,j,h] = w_h.(r_i-r_j) + w4_h|r_i-r_j|^2 decomposes as a contract-5
    matmul: lhsT[:,p] = [s_h*r_i - w_h ; w4_h ; A_ih] (s_h = -2*w4_h,
    A_ih = r_i.w_h + |r_i|^2 w4_h), rhs[:,j] = [r_j ; |r_j|^2 ; 1].
    Partitions pack 128 (i,h) pairs per tile; free axis is j (2048).
  - S[i,h] = sum_j sigmoid(z): Act-engine tiles use the Sigmoid activation
    with accum_out; Vector-engine tiles use a custom 8-stage DVE op
    (registered at runtime) computing f = zm * bitcast(~bits(|zm|+K)),
    zm = z(z^2+a) -- an odd, saturating rational sigmoid whose reciprocal
    comes from the bitwise-NOT seed trick -- with an ADD accumulator.
    sigma ~= 0.5 + 0.5*c*f; the affine correction is folded into W2cat rows
    and the output bias on the host, so DVE h-columns cost ONE instruction.
    Splitting the 192 (i,h)-tiles across both engines nearly doubles
    throughput vs. the act-engine-only roofline.

Sharding: i-axis split across 8 cores (256 rows each); every core holds the
full r for the j axis, no collectives. Output slices are concatenated on host.
"""

import numpy as np

E = 2048
NCORES = 8
EI = E // NCORES  # 256 rows per core
H = 96
NNUC = 64
NT = 2 * H  # 192 tiles of 128 (i,h) pairs per core

# Number of the 192 (i,h)-tiles computed on the Vector engine, spread evenly
# between Act-engine tiles so both consumer engines run concurrently off the
# two PSUM slots.
N_DVE_T = 56

# bf16 Schraudolph-sigmoid constants (validated bit-exactly on HW by
# probe_dve.py):  sigma(z) ~= 1/v,  v = min(1 + u, BIG),
#   u = bf16_bits(sat_i16(max(-C1SIG*(z - SHIFT), 0)))  ~ e^-z
#   sigma_bits = sat_i16(MAGIC - bits(v));  final max(.,0) flushes NaN.
C1SIG = 184.6627  # 128*log2(e)
SHIFT = 16256.0 / C1SIG  # exponent-bias offset, folded into the matmul A-row
MAGIC = 32512.0  # 2*16256: reciprocal bit-trick magic; sigma<=1 by construction
BIGF = 1.0e38

_CACHE = {}


def _dve_t_set():
    s = set()
    acc = 0
    for t in range(NT):
        acc += N_DVE_T
        if acc >= NT:
            acc -= NT
            s.add(t)
    assert len(s) == N_DVE_T
    return s


def _split_sync_waits(bir_json):
    """This walrus build accepts at most ONE sync wait per instruction
    (setupSyncWait: 'Too many sync wait commands'), while Tile freely attaches
    several. Rewrite the BIR: move all but one wait of each instruction onto
    single-wait NoOps on the same engine immediately before it — the engine's
    in-order sequencer makes this semantically identical."""
    import json

    m = json.loads(bir_json)
    ctr = 0
    for fn in m["functions"]:
        for blk in fn["blocks"]:
            out = []
            for inst in blk["instructions"]:
                si = inst.get("sync_info")
                waits = (si or {}).get("on_wait") or []
                if len(waits) > 1:
                    for w in waits[:-1]:
                        ctr += 1
                        out.append(
                            {
                                "debug": inst.get("debug", 0),
                                "engine": inst["engine"],
                                "ins": [],
                                "name": f"WSPLIT-{ctr}",
                                "opcode": "NoOp",
                                "outs": [],
                                "sync_info": {"on_update": [], "on_wait": [w]},
                            }
                        )
                    si["on_wait"] = [waits[-1]]
                out.append(inst)
            blk["instructions"] = out
    return json.dumps(m).encode()


def _install_compile_patch():
    if _CACHE.get("patched"):
        return
    import concourse.bass_utils as bu
    import concourse.bass2jax as b2j

    orig = bu.compile_bir_kernel

    def patched(bir_json, tmpdir, neff_name="file.neff"):
        return orig(_split_sync_waits(bir_json), tmpdir, neff_name)

    bu.compile_bir_kernel = patched
    b2j.compile_bir_kernel = patched
    _CACHE["patched"] = True


def _build():
    import concourse.bass as bass
    import concourse.tile as tile
    from concourse import mybir
    from concourse.vector_clock import ScopedClock, VectorClock

    f32 = mybir.dt.float32
    f16 = mybir.dt.float16
    bf16 = mybir.dt.bfloat16
    i16 = mybir.dt.int16
    AF = mybir.ActivationFunctionType
    ALU = mybir.AluOpType

    dve_t = _dve_t_set()

    class _TC(tile.TileContext):
        # This walrus build rejects instructions carrying more than ~2 sem
        # waits; the stock tail drain carries one per logical processor.
        # Split them into single-wait NOPs on the sync engine ahead of it.
        def _drain_and_barrier(self, tick_clock, wait_clock):
            gc = tick_clock.global_clock
            n = len(gc)
            for p in range(n):
                t = gc[p]
                if t > 0:
                    vec = [0] * n
                    vec[p] = t
                    nop = self.nc.sync.nop()
                    wait_clock.add_sem_waits(
                        nop.ins, ScopedClock({None: VectorClock(vec)})
                    )
            self.nc.sync.drain()
            self.nc.all_engine_barrier()
            popped = self.nc._tile_sem_poison_stack.pop()
            assert popped is self._sem_poison
            self.nc.clear_and_free_semaphores(list(self.sems.allocated().values()))
            self.nc.all_engine_barrier()

    nc = bass.Bass(name="gnn")
    PV = nc.dram_tensor("PV", [5, NT * 128], f16, kind="ExternalInput")
    AV = nc.dram_tensor("AV", [128, NT], f32, kind="ExternalInput")
    YD = nc.dram_tensor("YD", [5, E], f16, kind="ExternalInput")
    spin1 = nc.dram_tensor("spin1", [2, EI], f32, kind="ExternalInput")
    embA = nc.dram_tensor("embA", [128, EI], f32, kind="ExternalInput")
    embB = nc.dram_tensor("embB", [128, EI], f32, kind="ExternalInput")
    W2A = nc.dram_tensor("W2A", [H, 64], f32, kind="ExternalInput")
    W0A = nc.dram_tensor("W0A", [128, 64], f32, kind="ExternalInput")
    W0B = nc.dram_tensor("W0B", [128, 64], f32, kind="ExternalInput")
    W0C = nc.dram_tensor("W0C", [2, 64], f32, kind="ExternalInput")
    EYE = nc.dram_tensor("EYE", [128, 128], f32, kind="ExternalInput")
    out = nc.dram_tensor("out", [EI, 64], f32, kind="ExternalOutput")

    with _TC(nc) as tc:
        import contextlib

        with contextlib.ExitStack() as ctx:
            const = ctx.enter_context(tc.tile_pool(name="const", bufs=1))
            work = ctx.enter_context(tc.tile_pool(name="work", bufs=2))
            psum = ctx.enter_context(tc.tile_pool(name="psum", bufs=2, space="PSUM"))

            def load(dram, shape, name, dt=f32):
                t = const.tile(shape, dt, tag=name, name=name)
                nc.sync.dma_start(out=t, in_=dram[:, :])
                return t

            PV_sb = load(PV, [5, NT * 128], "PV", f16)
            AV_sb = load(AV, [128, NT], "AV")
            Y_sb = load(YD, [5, E], "YD", f16)
            spin1_sb = load(spin1, [2, EI], "spin1")
            embA_sb = load(embA, [128, EI], "embA")
            embB_sb = load(embB, [128, EI], "embB")
            W2A_sb = load(W2A, [H, 64], "W2A")
            W0A_sb = load(W0A, [128, 64], "W0A")
            W0B_sb = load(W0B, [128, 64], "W0B")
            W0C_sb = load(W0C, [2, 64], "W0C")
            EYE_sb = load(EYE, [128, 128], "EYE")

            # accumulator columns, one tile per engine per i-half (disjoint
            # h-columns; keeping engines on separate tiles avoids cross-engine
            # write ordering): S_*[half][:, h] = S[i = 128*half + p, h]
            S_act = [
                const.tile([128, H], f32, tag=f"Sa{q}", name=f"Sa{q}") for q in range(2)
            ]
            S_dve = [
                const.tile([128, H], f32, tag=f"Sd{q}", name=f"Sd{q}") for q in range(2)
            ]
            for q in range(2):
                nc.vector.memset(S_act[q], 0.0)
                nc.vector.memset(S_dve[q], 0.0)
            # scratch for the DVE sigmoid chain (engine-serial, reused)
            I16 = const.tile([128, E], i16, tag="I16", name="I16")
            Vb = const.tile([128, E], bf16, tag="Vb", name="Vb")
            SI = const.tile([128, E], i16, tag="SI", name="SI")
            H1 = const.tile([128, E // 2], f16, tag="H1", name="H1")
            H2 = const.tile([128, E // 4], f16, tag="H2", name="H2")
            JK = const.tile([128, E // 4], f16, tag="JK", name="JK")

            for t in range(NT):
                h = t // 2
                half = t % 2
                T_ps = psum.tile([128, E], f32, tag="ps", name="ps")
                lhsT = PV_sb[:, t * 128 : (t + 1) * 128]
                for c in range(4):
                    nc.tensor.matmul(
                        T_ps[:, c * 512 : (c + 1) * 512],
                        lhsT,
                        Y_sb[:, c * 512 : (c + 1) * 512],
                        start=True,
                        stop=True,
                    )
                if t in dve_t:
                    # P1: I16 = sat16(max(-C1SIG*zs, 0)); zs includes -SHIFT via PV row 4
                    nc.vector.tensor_scalar(
                        I16, T_ps, -C1SIG, 0.0, ALU.mult, ALU.max
                    )
                    # P2: V = min(u + 1, BIG); u = I16 as bf16 (NaN from +sat flushed)
                    nc.vector.tensor_scalar(
                        Vb, I16.bitcast(bf16), 1.0, BIGF, ALU.add, ALU.min
                    )
                    # P3: SI = sat16(MAGIC - bits(V)); sigma = SI as bf16, in [0,1]
                    nc.vector.tensor_scalar(
                        SI, Vb.bitcast(i16), -1.0, MAGIC, ALU.mult, ALU.add
                    )
                    # P4: pairwise-halving reduce at 2x (sigma values are
                    # NaN-free by construction), then 1x accumulate
                    sg = SI.bitcast(bf16)
                    nc.vector.tensor_add(H1, sg[:, 0 : E // 2], sg[:, E // 2 : E])
                    nc.vector.tensor_add(H2, H1[:, 0 : E // 4], H1[:, E // 4 : E // 2])
                    nc.vector.tensor_scalar(
                        JK,
                        H2,
                        1.0,
                        0.0,
                        ALU.mult,
                        ALU.add,
                        accum_out=S_dve[half][:, h : h + 1],
                    )
                else:
                    nc.scalar.activation(
                        out=T_ps,
                        in_=T_ps,
                        func=AF.Sigmoid,
                        bias=AV_sb[:, t : t + 1],
                        accum_out=S_act[half][:, h : h + 1],
                    )

            for half in range(2):
                isl = slice(half * 128, (half + 1) * 128)
                Ssum = work.tile([128, H], f32, tag="Ssum", name="Ssum")
                nc.vector.tensor_add(Ssum, S_act[half], S_dve[half])
                ST_ps = psum.tile([H, 128], f32, tag="ps", name="ps")
                nc.tensor.transpose(ST_ps, Ssum, EYE_sb)
                ST_sb = work.tile([H, 128], f32, tag="ST", name="ST")
                nc.vector.tensor_copy(ST_sb, ST_ps)
                O_ps = psum.tile([128, 64], f32, tag="ps", name="ps")
                nc.tensor.matmul(O_ps, ST_sb, W2A_sb, start=True, stop=False)
                nc.tensor.matmul(O_ps, embA_sb[:, isl], W0A_sb, start=False, stop=False)
                nc.tensor.matmul(O_ps, embB_sb[:, isl], W0B_sb, start=False, stop=False)
                nc.tensor.matmul(
                    O_ps, spin1_sb[:, isl], W0C_sb, start=False, stop=True
                )
                O_sb = work.tile([128, 64], f32, tag="O", name="O")
                nc.vector.tensor_copy(O_sb, O_ps)
                nc.sync.dma_start(out=out[isl, :], in_=O_sb)

    return nc


def _host_prep(r, R, W0, b0, W1s, W2s, n_up, n_down):
    r = np.asarray(r, np.float32)
    R = np.asarray(R, np.float32)
    W0 = np.asarray(W0, np.float32)
    b0 = np.asarray(b0, np.float32)
    W1s = np.asarray(W1s, np.float32)
    W2s = np.asarray(W2s, np.float32)
    n_up = int(n_up)
    dve_t = _dve_t_set()

    W1cat = np.concatenate([W1s[0], W1s[1], W1s[2]], axis=1)  # [4, 96]
    w4 = W1cat[3]  # [96]
    W2dev = np.concatenate([W2s[0], W2s[1], W2s[2]], axis=0).astype(np.float32)

    n2 = (r * r).sum(1).astype(np.float32)  # [E]
    A = (r @ W1cat[0:3] + n2[:, None] * w4[None, :]).astype(np.float32)  # [E, 96]

    # rhs Y [5, E] = [r ; |r|^2 ; 1]
    Y = np.concatenate(
        [r.T, n2[None, :], np.ones((1, E), np.float32)], axis=0
    ).astype(np.float16)

    # electron-nucleus features, host-computed in the on-device embT layout
    d_en = r[:, None, :] - R[None, :, :]  # [E, N, 3]
    dist = np.sqrt((d_en.astype(np.float64) ** 2).sum(-1))  # [E, N]
    log_d = np.log1p(dist)
    g = (log_d / dist).astype(np.float32)  # [E, N]
    rescaled = d_en * g[..., None]  # [E, N, 3]
    # embA rows: [rescaled_x(n) ; rescaled_y(n)], embB rows: [rescaled_z(n) ; log_d(n)]
    embA = np.concatenate([rescaled[:, :, 0].T, rescaled[:, :, 1].T], axis=0).astype(
        np.float32
    )  # [128, E]
    embB = np.concatenate([rescaled[:, :, 2].T, np.asarray(log_d.T, np.float32)], axis=0).astype(
        np.float32
    )  # [128, E]

    n_idx = np.arange(NNUC)
    perm_a = np.concatenate([3 * n_idx, 3 * n_idx + 1])
    perm_b = np.concatenate([3 * n_idx + 2, 192 + n_idx])
    W0A = W0[perm_a].astype(np.float32)
    W0B = W0[perm_b].astype(np.float32)
    W0C = np.stack([W0[256], b0]).astype(np.float32)

    spin = np.ones(E, np.float32)
    spin[n_up:] = -1.0
    spin1 = np.stack([spin, np.ones(E, np.float32)]).astype(np.float32)

    eye = np.eye(128, dtype=np.float32)

    shared = {
        "YD": Y,
        "W2A": W2dev,
        "W0A": W0A,
        "W0B": W0B,
        "W0C": W0C,
        "EYE": eye,
    }
    in_maps = []
    for c in range(NCORES):
        isl = slice(c * EI, (c + 1) * EI)
        rc = r[isl]  # [EI, 3]
        Ac = A[isl]  # [EI, 96]
        # PV [5, NT*128]: column 128*t + p  <->  (i = 128*(t%2)+p, h = t//2)
        PVc = np.zeros((5, NT * 128), np.float32)
        AVc = np.zeros((128, NT), np.float32)
        s_h = -2.0 * w4  # [96]
        # [H, EI, 3]: s_h * r_i - w_h
        P3 = s_h[:, None, None] * rc[None, :, :] - W1cat[0:3].T[:, None, :]
        for h in range(H):
            for half in range(2):
                t = 2 * h + half
                csl = slice(t * 128, (t + 1) * 128)
                ri = slice(half * 128, (half + 1) * 128)
                PVc[0:3, csl] = P3[h, ri].T
                PVc[3, csl] = w4[h]
                if t in dve_t:
                    # DVE tiles: bias folded into the matmul, pre-shifted by
                    # the Schraudolph exponent offset so P1 is mult+max only
                    PVc[4, csl] = Ac[ri, h] - SHIFT
                else:
                    AVc[:, t] = Ac[ri, h]
        m = dict(shared)
        m["PV"] = PVc.astype(np.float16)
        m["AV"] = AVc
        m["spin1"] = np.ascontiguousarray(spin1[:, isl])
        m["embA"] = np.ascontiguousarray(embA[:, isl])
        m["embB"] = np.ascontiguousarray(embB[:, isl])
        in_maps.append(m)
    return in_maps


def _get_runner():
    """Build the Bass program once and hold a single jitted shard_map
    executable so repeat kernel() calls skip retracing/recompiling.
    Mirrors concourse.bass2jax.run_bass_via_pjrt's multi-core path."""
    if "runner" in _CACHE:
        return _CACHE["runner"]

    import jax
    from jax.experimental.shard_map import shard_map
    from jax.sharding import Mesh, PartitionSpec

    from concourse import mybir
    from concourse.bass2jax import (
        _bass_exec_p,
        install_neuronx_cc_hook,
        partition_id_tensor,
    )

    _install_compile_patch()
    install_neuronx_cc_hook()
    nc = _CACHE.setdefault("nc", _build())

    partition_name = nc.partition_id_tensor.name if nc.partition_id_tensor else None
    in_names = []
    out_names = []
    out_avals = []
    zero_outs = []
    for alloc in nc.m.functions[0].allocations:
        if not isinstance(alloc, mybir.MemoryLocationSet):
            continue
        name = alloc.memorylocations[0].name
        if alloc.kind == "ExternalInput":
            if name != partition_name:
                in_names.append(name)
        elif alloc.kind == "ExternalOutput":
            shape = tuple(alloc.tensor_shape)
            dtype = mybir.dt.np(alloc.dtype)
            out_names.append(name)
            out_avals.append(jax.core.ShapedArray(shape, dtype))
            zero_outs.append(np.zeros(shape, dtype))
    n_params = len(in_names)
    n_outs = len(out_names)
    all_in_names = list(in_names) + list(out_names)
    if partition_name is not None:
        all_in_names.append(partition_name)
    donate = tuple(range(n_params, n_params + n_outs))

    def _body(*args):
        operands = list(args)
        if partition_name is not None:
            operands.append(partition_id_tensor())
        outs = _bass_exec_p.bind(
            *operands,
            out_avals=tuple(out_avals),
            in_names=tuple(all_in_names),
            out_names=tuple(out_names),
            lowering_input_output_aliases=(),
            sim_require_finite=True,
            sim_require_nnan=True,
            nc=nc,
        )
        return tuple(outs)

    devices = jax.devices()[:NCORES]
    mesh = Mesh(np.asarray(devices), ("core",))
    in_specs = (PartitionSpec("core"),) * (n_params + n_outs)
    out_specs = (PartitionSpec("core"),) * n_outs
    sharded = jax.jit(
        shard_map(
            _body, mesh=mesh, in_specs=in_specs, out_specs=out_specs, check_rep=False
        ),
        donate_argnums=donate,
        keep_unused=True,
    )

    def runner(in_maps):
        concat_in = [
            np.concatenate([np.asarray(in_maps[c][n]) for c in range(NCORES)], axis=0)
            for n in in_names
        ]
        concat_zeros = [
            np.zeros((NCORES * z.shape[0], *z.shape[1:]), z.dtype) for z in zero_outs
        ]
        out_arrs = sharded(*concat_in, *concat_zeros)
        return np.asarray(out_arrs[out_names.index("out")])

    _CACHE["runner"] = runner
    return runner


def kernel(r, R, W0, b0, W1s, W2s, n_up, n_down):
    runner = _get_runner()
    in_maps = _host_prep(r, R, W0, b0, W1s, W2s, n_up, n_down)
    return runner(in_maps)


# revision 22
# speedup vs baseline: 1.1885x; 1.1885x over previous
"""Bass/Trainium2 kernel for the NaiveGNN message-passing problem.

Math (reference): h = emb @ W0 + b0 + sum_l (sum_j sigmoid(ee @ W1s[l])) @ W2s[l]
with ee[i,j] = [r_i - r_j, |r_i - r_j|^2].

Key identities:
  - The three layers share the ee tensor, so W1s concatenates to W1cat [4,96]
    and W2s to W2cat [96,64]: one fused pass with H=96 sigmoid features.
  - z[i,j,h] = w_h.(r_i-r_j) + w4_h|r_i-r_j|^2 decomposes as a contract-5
    matmul: lhsT[:,p] = [s_h*r_i - w_h ; w4_h ; A_ih] (s_h = -2*w4_h,
    A_ih = r_i.w_h + |r_i|^2 w4_h), rhs[:,j] = [r_j ; |r_j|^2 ; 1].
    Partitions pack 128 (i,h) pairs per tile; free axis is j (2048).
  - S[i,h] = sum_j sigmoid(z): Act-engine tiles use the Sigmoid activation
    with accum_out; Vector-engine tiles use a custom 8-stage DVE op
    (registered at runtime) computing f = zm * bitcast(~bits(|zm|+K)),
    zm = z(z^2+a) -- an odd, saturating rational sigmoid whose reciprocal
    comes from the bitwise-NOT seed trick -- with an ADD accumulator.
    sigma ~= 0.5 + 0.5*c*f; the affine correction is folded into W2cat rows
    and the output bias on the host, so DVE h-columns cost ONE instruction.
    Splitting the 192 (i,h)-tiles across both engines nearly doubles
    throughput vs. the act-engine-only roofline.

Sharding: i-axis split across 8 cores (256 rows each); every core holds the
full r for the j axis, no collectives. Output slices are concatenated on host.
"""

import numpy as np

E = 2048
NCORES = 8
EI = E // NCORES  # 256 rows per core
H = 96
NNUC = 64
NT = 2 * H  # 192 tiles of 128 (i,h) pairs per core

# Number of the 192 (i,h)-tiles computed on the Vector engine, spread evenly
# between Act-engine tiles so both consumer engines run concurrently off the
# two PSUM slots.
N_DVE_T = 48

# bf16 Schraudolph-sigmoid constants (validated bit-exactly on HW by
# probe_dve.py):  sigma(z) ~= 1/v,  v = min(1 + u, BIG),
#   u = bf16_bits(sat_i16(max(-C1SIG*(z - SHIFT), 0)))  ~ e^-z
#   sigma_bits = sat_i16(MAGIC - bits(v));  final max(.,0) flushes NaN.
C1SIG = 184.6627  # 128*log2(e)
# exponent-bias offset folded into the matmul A-row; +0.17 centers the
# Schraudolph sawtooth so the mean sigma error on the real z-distribution
# is ~0 (tune_shift.py: S-err mean -0.04, worst-case rel err 7.3e-3)
SHIFT = 16256.0 / C1SIG + 0.17
MAGIC = 32512.0  # 2*16256: reciprocal bit-trick magic; sigma<=1 by construction
BIGF = 1.0e38

_CACHE = {}


def _dve_t_set():
    s = set()
    acc = 0
    for t in range(NT):
        acc += N_DVE_T
        if acc >= NT:
            acc -= NT
            s.add(t)
    assert len(s) == N_DVE_T
    return s


def _split_sync_waits(bir_json):
    """This walrus build accepts at most ONE sync wait per instruction
    (setupSyncWait: 'Too many sync wait commands'), while Tile freely attaches
    several. Rewrite the BIR: move all but one wait of each instruction onto
    single-wait NoOps on the same engine immediately before it — the engine's
    in-order sequencer makes this semantically identical."""
    import json

    m = json.loads(bir_json)
    ctr = 0
    for fn in m["functions"]:
        for blk in fn["blocks"]:
            out = []
            for inst in blk["instructions"]:
                si = inst.get("sync_info")
                waits = (si or {}).get("on_wait") or []
                if len(waits) > 1:
                    for w in waits[:-1]:
                        ctr += 1
                        out.append(
                            {
                                "debug": inst.get("debug", 0),
                                "engine": inst["engine"],
                                "ins": [],
                                "name": f"WSPLIT-{ctr}",
                                "opcode": "NoOp",
                                "outs": [],
                                "sync_info": {"on_update": [], "on_wait": [w]},
                            }
                        )
                    si["on_wait"] = [waits[-1]]
                out.append(inst)
            blk["instructions"] = out
    return json.dumps(m).encode()


def _install_compile_patch():
    if _CACHE.get("patched"):
        return
    import concourse.bass_utils as bu
    import concourse.bass2jax as b2j

    orig = bu.compile_bir_kernel

    def patched(bir_json, tmpdir, neff_name="file.neff"):
        return orig(_split_sync_waits(bir_json), tmpdir, neff_name)

    bu.compile_bir_kernel = patched
    b2j.compile_bir_kernel = patched
    _CACHE["patched"] = True


def _build():
    import concourse.bass as bass
    import concourse.tile as tile
    from concourse import mybir
    from concourse.vector_clock import ScopedClock, VectorClock

    f32 = mybir.dt.float32
    f16 = mybir.dt.float16
    bf16 = mybir.dt.bfloat16
    i16 = mybir.dt.int16
    AF = mybir.ActivationFunctionType
    ALU = mybir.AluOpType

    dve_t = _dve_t_set()

    class _TC(tile.TileContext):
        # This walrus build rejects instructions carrying more than ~2 sem
        # waits; the stock tail drain carries one per logical processor.
        # Split them into single-wait NOPs on the sync engine ahead of it.
        def _drain_and_barrier(self, tick_clock, wait_clock):
            gc = tick_clock.global_clock
            n = len(gc)
            for p in range(n):
                t = gc[p]
                if t > 0:
                    vec = [0] * n
                    vec[p] = t
                    nop = self.nc.sync.nop()
                    wait_clock.add_sem_waits(
                        nop.ins, ScopedClock({None: VectorClock(vec)})
                    )
            self.nc.sync.drain()
            self.nc.all_engine_barrier()
            popped = self.nc._tile_sem_poison_stack.pop()
            assert popped is self._sem_poison
            self.nc.clear_and_free_semaphores(list(self.sems.allocated().values()))
            self.nc.all_engine_barrier()

    nc = bass.Bass(name="gnn")
    PV = nc.dram_tensor("PV", [5, NT * 128], f16, kind="ExternalInput")
    AV = nc.dram_tensor("AV", [128, NT], f32, kind="ExternalInput")
    YD = nc.dram_tensor("YD", [5, E], f16, kind="ExternalInput")
    spin1 = nc.dram_tensor("spin1", [2, EI], f32, kind="ExternalInput")
    embA = nc.dram_tensor("embA", [128, EI], f32, kind="ExternalInput")
    embB = nc.dram_tensor("embB", [128, EI], f32, kind="ExternalInput")
    W2A = nc.dram_tensor("W2A", [H, 64], f32, kind="ExternalInput")
    W0A = nc.dram_tensor("W0A", [128, 64], f32, kind="ExternalInput")
    W0B = nc.dram_tensor("W0B", [128, 64], f32, kind="ExternalInput")
    W0C = nc.dram_tensor("W0C", [2, 64], f32, kind="ExternalInput")
    EYE = nc.dram_tensor("EYE", [128, 128], f32, kind="ExternalInput")
    out = nc.dram_tensor("out", [EI, 64], f32, kind="ExternalOutput")

    with _TC(nc) as tc:
        import contextlib

        with contextlib.ExitStack() as ctx:
            const = ctx.enter_context(tc.tile_pool(name="const", bufs=1))
            work = ctx.enter_context(tc.tile_pool(name="work", bufs=2))
            psum = ctx.enter_context(tc.tile_pool(name="psum", bufs=2, space="PSUM"))

            def load(dram, shape, name, dt=f32):
                t = const.tile(shape, dt, tag=name, name=name)
                nc.sync.dma_start(out=t, in_=dram[:, :])
                return t

            PV_sb = load(PV, [5, NT * 128], "PV", f16)
            AV_sb = load(AV, [128, NT], "AV")
            Y_sb = load(YD, [5, E], "YD", f16)
            spin1_sb = load(spin1, [2, EI], "spin1")
            embA_sb = load(embA, [128, EI], "embA")
            embB_sb = load(embB, [128, EI], "embB")
            W2A_sb = load(W2A, [H, 64], "W2A")
            W0A_sb = load(W0A, [128, 64], "W0A")
            W0B_sb = load(W0B, [128, 64], "W0B")
            W0C_sb = load(W0C, [2, 64], "W0C")
            EYE_sb = load(EYE, [128, 128], "EYE")

            # accumulator columns, one tile per engine per i-half (disjoint
            # h-columns; keeping engines on separate tiles avoids cross-engine
            # write ordering): S_*[half][:, h] = S[i = 128*half + p, h]
            S_act = [
                const.tile([128, H], f32, tag=f"Sa{q}", name=f"Sa{q}") for q in range(2)
            ]
            S_dve = [
                const.tile([128, H], f32, tag=f"Sd{q}", name=f"Sd{q}") for q in range(2)
            ]
            for q in range(2):
                nc.vector.memset(S_act[q], 0.0)
                nc.vector.memset(S_dve[q], 0.0)
            # scratch for the DVE sigmoid chain (engine-serial, reused)
            I16 = const.tile([128, E], i16, tag="I16", name="I16")
            Vb = const.tile([128, E], bf16, tag="Vb", name="Vb")
            SI = const.tile([128, E], i16, tag="SI", name="SI")
            H1 = const.tile([128, E // 2], f16, tag="H1", name="H1")
            H2 = const.tile([128, E // 4], f16, tag="H2", name="H2")
            JK = const.tile([128, E // 4], f16, tag="JK", name="JK")

            for t in range(NT):
                h = t // 2
                half = t % 2
                T_ps = psum.tile([128, E], f32, tag="ps", name="ps")
                lhsT = PV_sb[:, t * 128 : (t + 1) * 128]
                for c in range(4):
                    nc.tensor.matmul(
                        T_ps[:, c * 512 : (c + 1) * 512],
                        lhsT,
                        Y_sb[:, c * 512 : (c + 1) * 512],
                        start=True,
                        stop=True,
                    )
                if t in dve_t:
                    # P1: I16 = sat16(max(-C1SIG*zs, 0)); zs includes -SHIFT via PV row 4
                    nc.vector.tensor_scalar(
                        I16, T_ps, -C1SIG, 0.0, ALU.mult, ALU.max
                    )
                    # P2: V = min(u + 1, BIG); u = I16 as bf16 (NaN from +sat flushed)
                    nc.vector.tensor_scalar(
                        Vb, I16.bitcast(bf16), 1.0, BIGF, ALU.add, ALU.min
                    )
                    # P3: SI = sat16(MAGIC - bits(V)); sigma = SI as bf16, in [0,1]
                    nc.vector.tensor_scalar(
                        SI, Vb.bitcast(i16), -1.0, MAGIC, ALU.mult, ALU.add
                    )
                    # P4: pairwise-halving reduce at 2x (sigma values are
                    # NaN-free by construction), then 1x accumulate
                    sg = SI.bitcast(bf16)
                    nc.vector.tensor_add(H1, sg[:, 0 : E // 2], sg[:, E // 2 : E])
                    nc.vector.tensor_add(H2, H1[:, 0 : E // 4], H1[:, E // 4 : E // 2])
                    nc.vector.tensor_scalar(
                        JK,
                        H2,
                        1.0,
                        0.0,
                        ALU.mult,
                        ALU.add,
                        accum_out=S_dve[half][:, h : h + 1],
                    )
                else:
                    nc.scalar.activation(
                        out=T_ps,
                        in_=T_ps,
                        func=AF.Sigmoid,
                        bias=AV_sb[:, t : t + 1],
                        accum_out=S_act[half][:, h : h + 1],
                    )

            for half in range(2):
                isl = slice(half * 128, (half + 1) * 128)
                Ssum = work.tile([128, H], f32, tag="Ssum", name="Ssum")
                nc.vector.tensor_add(Ssum, S_act[half], S_dve[half])
                ST_ps = psum.tile([H, 128], f32, tag="ps", name="ps")
                nc.tensor.transpose(ST_ps, Ssum, EYE_sb)
                ST_sb = work.tile([H, 128], f32, tag="ST", name="ST")
                nc.vector.tensor_copy(ST_sb, ST_ps)
                O_ps = psum.tile([128, 64], f32, tag="ps", name="ps")
                nc.tensor.matmul(O_ps, ST_sb, W2A_sb, start=True, stop=False)
                nc.tensor.matmul(O_ps, embA_sb[:, isl], W0A_sb, start=False, stop=False)
                nc.tensor.matmul(O_ps, embB_sb[:, isl], W0B_sb, start=False, stop=False)
                nc.tensor.matmul(
                    O_ps, spin1_sb[:, isl], W0C_sb, start=False, stop=True
                )
                O_sb = work.tile([128, 64], f32, tag="O", name="O")
                nc.vector.tensor_copy(O_sb, O_ps)
                nc.sync.dma_start(out=out[isl, :], in_=O_sb)

    return nc


def _host_prep(r, R, W0, b0, W1s, W2s, n_up, n_down):
    r = np.asarray(r, np.float32)
    R = np.asarray(R, np.float32)
    W0 = np.asarray(W0, np.float32)
    b0 = np.asarray(b0, np.float32)
    W1s = np.asarray(W1s, np.float32)
    W2s = np.asarray(W2s, np.float32)
    n_up = int(n_up)
    dve_t = _dve_t_set()

    W1cat = np.concatenate([W1s[0], W1s[1], W1s[2]], axis=1)  # [4, 96]
    w4 = W1cat[3]  # [96]
    W2dev = np.concatenate([W2s[0], W2s[1], W2s[2]], axis=0).astype(np.float32)

    n2 = (r * r).sum(1).astype(np.float32)  # [E]
    A = (r @ W1cat[0:3] + n2[:, None] * w4[None, :]).astype(np.float32)  # [E, 96]

    # rhs Y [5, E] = [r ; |r|^2 ; 1]
    Y = np.concatenate(
        [r.T, n2[None, :], np.ones((1, E), np.float32)], axis=0
    ).astype(np.float16)

    # electron-nucleus features, host-computed in the on-device embT layout
    d_en = r[:, None, :] - R[None, :, :]  # [E, N, 3]
    dist = np.sqrt((d_en.astype(np.float64) ** 2).sum(-1))  # [E, N]
    log_d = np.log1p(dist)
    g = (log_d / dist).astype(np.float32)  # [E, N]
    rescaled = d_en * g[..., None]  # [E, N, 3]
    # embA rows: [rescaled_x(n) ; rescaled_y(n)], embB rows: [rescaled_z(n) ; log_d(n)]
    embA = np.concatenate([rescaled[:, :, 0].T, rescaled[:, :, 1].T], axis=0).astype(
        np.float32
    )  # [128, E]
    embB = np.concatenate([rescaled[:, :, 2].T, np.asarray(log_d.T, np.float32)], axis=0).astype(
        np.float32
    )  # [128, E]

    n_idx = np.arange(NNUC)
    perm_a = np.concatenate([3 * n_idx, 3 * n_idx + 1])
    perm_b = np.concatenate([3 * n_idx + 2, 192 + n_idx])
    W0A = W0[perm_a].astype(np.float32)
    W0B = W0[perm_b].astype(np.float32)
    W0C = np.stack([W0[256], b0]).astype(np.float32)

    spin = np.ones(E, np.float32)
    spin[n_up:] = -1.0
    spin1 = np.stack([spin, np.ones(E, np.float32)]).astype(np.float32)

    eye = np.eye(128, dtype=np.float32)

    shared = {
        "YD": Y,
        "W2A": W2dev,
        "W0A": W0A,
        "W0B": W0B,
        "W0C": W0C,
        "EYE": eye,
    }
    in_maps = []
    for c in range(NCORES):
        isl = slice(c * EI, (c + 1) * EI)
        rc = r[isl]  # [EI, 3]
        Ac = A[isl]  # [EI, 96]
        # PV [5, NT*128]: column 128*t + p  <->  (i = 128*(t%2)+p, h = t//2)
        PVc = np.zeros((5, NT * 128), np.float32)
        AVc = np.zeros((128, NT), np.float32)
        s_h = -2.0 * w4  # [96]
        # [H, EI, 3]: s_h * r_i - w_h
        P3 = s_h[:, None, None] * rc[None, :, :] - W1cat[0:3].T[:, None, :]
        for h in range(H):
            for half in range(2):
                t = 2 * h + half
                csl = slice(t * 128, (t + 1) * 128)
                ri = slice(half * 128, (half + 1) * 128)
                PVc[0:3, csl] = P3[h, ri].T
                PVc[3, csl] = w4[h]
                if t in dve_t:
                    # DVE tiles: bias folded into the matmul, pre-shifted by
                    # the Schraudolph exponent offset so P1 is mult+max only
                    PVc[4, csl] = Ac[ri, h] - SHIFT
                else:
                    AVc[:, t] = Ac[ri, h]
        m = dict(shared)
        m["PV"] = PVc.astype(np.float16)
        m["AV"] = AVc
        m["spin1"] = np.ascontiguousarray(spin1[:, isl])
        m["embA"] = np.ascontiguousarray(embA[:, isl])
        m["embB"] = np.ascontiguousarray(embB[:, isl])
        in_maps.append(m)
    return in_maps


def _get_runner():
    """Build the Bass program once and hold a single jitted shard_map
    executable so repeat kernel() calls skip retracing/recompiling.
    Mirrors concourse.bass2jax.run_bass_via_pjrt's multi-core path."""
    if "runner" in _CACHE:
        return _CACHE["runner"]

    import jax
    from jax.experimental.shard_map import shard_map
    from jax.sharding import Mesh, PartitionSpec

    from concourse import mybir
    from concourse.bass2jax import (
        _bass_exec_p,
        install_neuronx_cc_hook,
        partition_id_tensor,
    )

    _install_compile_patch()
    install_neuronx_cc_hook()
    nc = _CACHE.setdefault("nc", _build())

    partition_name = nc.partition_id_tensor.name if nc.partition_id_tensor else None
    in_names = []
    out_names = []
    out_avals = []
    zero_outs = []
    for alloc in nc.m.functions[0].allocations:
        if not isinstance(alloc, mybir.MemoryLocationSet):
            continue
        name = alloc.memorylocations[0].name
        if alloc.kind == "ExternalInput":
            if name != partition_name:
                in_names.append(name)
        elif alloc.kind == "ExternalOutput":
            shape = tuple(alloc.tensor_shape)
            dtype = mybir.dt.np(alloc.dtype)
            out_names.append(name)
            out_avals.append(jax.core.ShapedArray(shape, dtype))
            zero_outs.append(np.zeros(shape, dtype))
    n_params = len(in_names)
    n_outs = len(out_names)
    all_in_names = list(in_names) + list(out_names)
    if partition_name is not None:
        all_in_names.append(partition_name)
    donate = tuple(range(n_params, n_params + n_outs))

    def _body(*args):
        operands = list(args)
        if partition_name is not None:
            operands.append(partition_id_tensor())
        outs = _bass_exec_p.bind(
            *operands,
            out_avals=tuple(out_avals),
            in_names=tuple(all_in_names),
            out_names=tuple(out_names),
            lowering_input_output_aliases=(),
            sim_require_finite=True,
            sim_require_nnan=True,
            nc=nc,
        )
        return tuple(outs)

    devices = jax.devices()[:NCORES]
    mesh = Mesh(np.asarray(devices), ("core",))
    in_specs = (PartitionSpec("core"),) * (n_params + n_outs)
    out_specs = (PartitionSpec("core"),) * n_outs
    sharded = jax.jit(
        shard_map(
            _body, mesh=mesh, in_specs=in_specs, out_specs=out_specs, check_rep=False
        ),
        donate_argnums=donate,
        keep_unused=True,
    )

    def runner(in_maps):
        concat_in = [
            np.concatenate([np.asarray(in_maps[c][n]) for c in range(NCORES)], axis=0)
            for n in in_names
        ]
        concat_zeros = [
            np.zeros((NCORES * z.shape[0], *z.shape[1:]), z.dtype) for z in zero_outs
        ]
        out_arrs = sharded(*concat_in, *concat_zeros)
        return np.asarray(out_arrs[out_names.index("out")])

    _CACHE["runner"] = runner
    return runner


def kernel(r, R, W0, b0, W1s, W2s, n_up, n_down):
    runner = _get_runner()
    in_maps = _host_prep(r, R, W0, b0, W1s, W2s, n_up, n_down)
    return runner(in_maps)
